# revision 2
# baseline (speedup 1.0000x reference)
"""Trainium2 Bass kernel for the masked depth-binned 3x3 conv (Conv2.5D), v2.

Contract: kernel(**inputs) takes the FULL numpy inputs
  x     [8, 128, 64, 64] f32
  depth [8, 1, 64, 64]   f32
  fx    [8]              f32
  w0/w1/w2 [128, 128, 3, 3] f32
and returns the full output [8, 128, 64, 64] f32.

Data-parallel over N across the 8 NeuronCores (one sample per core).
Per core the op is decomposed as shifted 1x1 matmuls accumulated in PSUM
with the 3 depth bins folded into a Vandermonde moments basis (codes
b0=+1, b1=-1, b2=+2, none=0; see _prep_weights).

v2 changes vs v1 (all aimed at balancing the 4 engines near ~44us):
 - flat unpadded x layout: x arrives fp16 [C, 4096] from the host and is
   placed inside a [C, 66+4096+66] strip whose 66-wide borders are
   zeroed. Tap shifts become 1-D offsets; row-wrap columns are corrected
   by the selector being exactly 0 wherever the tap's depth sample is 0
   (bound clamp below), so no per-row padding is needed.
 - step-function selector: T = 2*[d>=lo2] - 3*[d>=lo1] + 2*[d>=lo0]
   - [d>hi0] with lo2 clamped to +eps gives the branch code field in 4
   comparisons + 3 scalar_tensor_tensor ops (intervals are adjacent so
   the indicator algebra is linear, no logical_ands needed).
 - moment multiplies stay on DVE as [C, 2*4096] pair-batched
   tensor_tensor ops (GpSimd tensor_tensor measured ~4x slower than its
   paper rate, so offloading to it loses; scalar_tensor_tensor and bf16
   forms also measured slower).
 - host-side prep: fp16 x, fp16 pre-transposed weights [C, 25*O],
   per-sample 0.5/fx replicated [64,1], depth pre-padded+row-shifted so
   the three row views land on partition bases 0/64/0 with no on-device
   shuffling.
 - fp16 output (host casts back to f32; quantization ~2e-4 << 2e-2).
"""

import numpy as np

import concourse.bass as bass
import concourse.mybir as mybir
import concourse.bacc as bacc
import concourse.tile as tile
from concourse.bass_utils import run_bass_kernel_spmd

F32 = mybir.dt.float32
F16 = mybir.dt.float16
AF = mybir.ActivationFunctionType
OP = mybir.AluOpType

N, C, O, H, W = 8, 128, 128, 64, 64
L = H * W                    # 4096
EDGE = 66                    # flat-strip zero border (max |shift| = 65)
XW = EDGE + L + EDGE         # 4228
NT = 8                       # psum column tiles
NTW = L // NT                # 512
# off-center taps; even-shift taps (dx==1) first so the shifted copy xb
# (needed by odd-shift taps) can be built while taps 1/7 are processed
KSO = (1, 7, 0, 2, 3, 5, 6, 8)
NK = len(KSO)
NMM = 1 + 3 * NK             # accumulation group length (center + moments)


def _build_program(loop_n=None, ablate=(), opts=()):
    """loop_n: if set, wrap the whole per-sample body in an on-device
    For_i loop (used only for timing measurements).
    ablate: timing-diagnostic switches ("bcast", "mult", "mm", "prec",
    "w1s") that remove pieces of the pipeline (results become wrong).
    opts: "stt" switches the moment multiply instruction form."""
    cd = L  # moment multiplies all on DVE (GpSimd measured far too slow)

    nc = bacc.Bacc("TRN2", target_bir_lowering=False, debug=False)

    x_in = nc.dram_tensor("x_in", [C, L], F16, kind="ExternalInput")
    # three row-shifted padded-depth planes + the 0.5/fx column, packed
    # into one tensor so the whole selector input is a single DMA
    d_in = nc.dram_tensor("d_in", [64, 3 * EDGE + 1], F32, kind="ExternalInput")
    w_in = nc.dram_tensor("w_in", [C, NMM * O], F16, kind="ExternalInput")
    out_d = nc.dram_tensor("out", [O, L], F16, kind="ExternalOutput")

    with tile.TileContext(nc) as tc:
        with (
            tc.tile_pool(name="wts", bufs=2) as wpool,
            tc.tile_pool(name="xbuf", bufs=2) as xpool,
            tc.tile_pool(name="sels", bufs=1) as spool,
            tc.tile_pool(name="selp", bufs=3) as selpool,
            tc.tile_pool(name="rowp", bufs=2, space="DRAM") as rowpool,
            tc.tile_pool(name="momd", bufs=5) as mdpool,
            tc.tile_pool(name="outb", bufs=1) as opool,
            tc.tile_pool(name="psum", bufs=1, space="PSUM") as ppool,
        ):
          with (tc.For_i(0, loop_n, 1)
                if loop_n is not None
                else __import__("contextlib").nullcontext()):
            # ---- input DMAs --------------------------------------------
            dsb = spool.tile([64, 3 * EDGE + 1], F32, tag="dsb")
            nc.sync.dma_start(out=dsb[:, :], in_=d_in[:, :])
            fxh = dsb[:, 3 * EDGE : 3 * EDGE + 1]

            xa = xpool.tile([C, XW], F16, tag="xa")
            nc.sync.dma_start(out=xa[:, EDGE : EDGE + L], in_=x_in[:, :])
            nc.vector.memset(xa[:, 0:EDGE], 0.0)
            nc.vector.memset(xa[:, EDGE + L : XW], 0.0)

            w_sb = wpool.tile([C, NMM * O], F16, tag="w")
            nc.scalar.dma_start(out=w_sb[:, :], in_=w_in[:, :])

            # ---- selector ----------------------------------------------
            # depth rows: drow[dy][p, :] = padded_depth[p + dy, :]
            drow = tuple(
                dsb[:, dy * EDGE : (dy + 1) * EDGE] for dy in range(3)
            )
            cview = dsb[:, EDGE + 1 : EDGE + 65]   # center depth [64, 64]

            h = spool.tile([64, 64], F32, tag="h")
            lo0 = spool.tile([64, 64], F32, tag="lo0")
            hi0 = spool.tile([64, 64], F32, tag="hi0")
            lo1 = spool.tile([64, 64], F32, tag="lo1")
            lo2 = spool.tile([64, 64], F32, tag="lo2")
            # h = half-bin = center * (0.5/fx); branch intervals are
            # [c+h, c+3h], [c-h, c+h], [c-3h, c-h]
            nc.vector.tensor_scalar(
                out=h[:, :], in0=cview, scalar1=fxh, scalar2=None,
                op0=OP.mult,
            )
            nc.vector.tensor_tensor(out=lo0[:, :], in0=cview, in1=h[:, :], op=OP.add)
            nc.vector.scalar_tensor_tensor(
                out=hi0[:, :], in0=h[:, :], scalar=2.0, in1=lo0[:, :],
                op0=OP.mult, op1=OP.add,
            )
            nc.vector.tensor_tensor(
                out=lo1[:, :], in0=cview, in1=h[:, :], op=OP.subtract
            )
            nc.vector.scalar_tensor_tensor(
                out=lo2[:, :], in0=h[:, :], scalar=-2.0, in1=lo1[:, :],
                op0=OP.mult, op1=OP.add,
            )
            # clamp lo2 to a positive eps: tap positions with depth==0
            # (outside the image) then fail every >= test and get code 0,
            # which also makes row-wrapped x reads in the flat layout
            # harmless (their moment is x*0)
            nc.vector.tensor_scalar(
                out=lo2[:, :], in0=lo2[:, :], scalar1=1e-30, scalar2=None,
                op0=OP.max,
            )

            # tap depth planes, assembled on ACT to keep DVE free
            d8 = spool.tile([64, NK * 64], F32, tag="d8")
            for i, k in enumerate(KSO):
                dy, dx = k // 3, k % 3
                nc.scalar.activation(
                    out=d8[:, i * 64 : (i + 1) * 64],
                    in_=drow[dy][:, dx : dx + 64],
                    func=AF.Copy,
                )

            selk = spool.tile([64, NK * 64], F16, tag="selk")
            if "prec" in ablate:
                nc.vector.memset(selk[:, :], 1.0)
            else:
                d8v = d8[:, :].rearrange("p (t x) -> p t x", x=64)

                def rep(f):
                    return f[:, :].unsqueeze(1).broadcast_to([64, NK, 64])

                s2 = spool.tile([64, NK * 64], F16, tag="s2")
                s1 = spool.tile([64, NK * 64], F16, tag="s1")
                s0 = spool.tile([64, NK * 64], F16, tag="s0")
                nh = spool.tile([64, NK * 64], F16, tag="nh")
                # comparisons must run on DVE: the Pool engine's ISA has no
                # is_ge/is_gt tensor_tensor opcodes (walrus rejects them)
                cmp_eng = nc.vector
                for s, f, op in (
                    (s2, lo2, OP.is_ge),
                    (s1, lo1, OP.is_ge),
                    (s0, lo0, OP.is_ge),
                    (nh, hi0, OP.is_gt),
                ):
                    cmp_eng.tensor_tensor(
                        out=s[:, :].rearrange("p (t x) -> p t x", x=64),
                        in0=d8v, in1=rep(f), op=op,
                    )
                # adjacent intervals => linear indicator algebra:
                # T = 2*s2 - 3*s1 + 2*s0 - nh  in {0, +1, -1, +2}
                # (balanced tree: ta, tc independent; selk = 2*tc + ta)
                ta = spool.tile([64, NK * 64], F16, tag="ta")
                tc = spool.tile([64, NK * 64], F16, tag="tc")
                nc.vector.scalar_tensor_tensor(
                    out=ta[:, :], in0=s2[:, :], scalar=2.0, in1=nh[:, :],
                    op0=OP.mult, op1=OP.subtract,
                )
                nc.vector.scalar_tensor_tensor(
                    out=tc[:, :], in0=s1[:, :], scalar=-1.5, in1=s0[:, :],
                    op0=OP.mult, op1=OP.add,
                )
                nc.vector.scalar_tensor_tensor(
                    out=selk[:, :], in0=tc[:, :], scalar=2.0, in1=ta[:, :],
                    op0=OP.mult, op1=OP.add,
                )

            # pack selector planes [64, 8*64] sbuf -> [8, L] dram rows
            row8 = rowpool.tile([NK, L], F16, tag="selrow")
            nc.sync.dma_start(
                out=bass.AP(
                    row8.tensor,
                    row8[:, :].offset,
                    [[64, 64], [L, NK], [1, 64]],
                ),
                in_=selk[:, :].rearrange("p (t x) -> p t x", x=64),
            )

            # shifted x copy for odd-offset taps (keeps DVE 2x alignment);
            # on ACT so DVE stays free. Needed first at tap slot 2.
            xb = xpool.tile([C, XW + 1], F16, tag="xb")
            nc.scalar.activation(out=xb[:, 1 : XW + 1], in_=xa[:, :], func=AF.Copy)

            # ---- matmul pipeline ---------------------------------------
            nt_eff = 1 if "mm" in ablate else NT
            psums = [
                ppool.tile([O, NTW], F32, tag=f"ps{t}", name=f"ps{t}")
                for t in range(nt_eff)
            ]
            # center tap: always branch 1, no masking
            for t in range(nt_eff):
                nc.tensor.matmul(
                    psums[t][:, :],
                    w_sb[:, 0:O],
                    xa[:, EDGE + NTW * t : EDGE + NTW * (t + 1)],
                    start=True,
                    stop=False,
                )

            if "bcast" in ablate:
                sel_const = selpool.tile([C, 2 * L], F16, tag="selc")
                nc.vector.memset(sel_const[:, :], 1.0)

            def emit_matmuls(idx, uD, uP, stop):
                """Accumulate w[:, idx] @ u for all psum tiles; the moment
                is split into uD (cols [0,cd)) and uP (cols [cd,L))."""
                if "w1s" in ablate:
                    idx = 0  # single stationary: isolates LdWeights cost
                for t in range(nt_eff):
                    c0, c1 = NTW * t, NTW * (t + 1)
                    segs = []
                    if c0 < cd:
                        segs.append((uD, c0, min(c1, cd)))
                    if c1 > cd and uP is not None:
                        segs.append((uP, max(c0, cd), c1))
                    for si, (u, a, b) in enumerate(segs):
                        off = 0 if u is uD else cd
                        nc.tensor.matmul(
                            psums[t][:, a - c0 : b - c0],
                            w_sb[:, idx * O : (idx + 1) * O],
                            u[:, a - off : b - off],
                            start=False,
                            stop=stop and (si == len(segs) - 1),
                        )

            # per-slot x views; broadcasts and moments are PAIRED (two taps
            # per DMA and per DVE op) to halve fixed per-op overheads. KSO
            # is ordered so both taps of a pair live in the same x buffer.
            NP = NK // 2
            xvs = []
            for i, k in enumerate(KSO):
                dy, dx = k // 3, k % 3
                shift = (dy - 1) * W + (dx - 1)
                if shift % 2 == 0:
                    xvs.append(xa[:, EDGE + shift : EDGE + shift + L])
                else:
                    xvs.append(xb[:, 1 + EDGE + shift : 1 + EDGE + shift + L])

            sel2s = [None] * NP

            def emit_bcast(p):
                if "bcast" in ablate:
                    sel2s[p] = sel_const
                    return
                sel2 = selpool.tile([C, 2 * L], F16, tag="sel2")
                eng = (nc.sync, nc.scalar)[p % 2]
                eng.dma_start(
                    out=sel2[:, :],
                    in_=bass.AP(
                        row8.tensor, row8[2 * p : 2 * p + 2, :].offset,
                        [[1, 2 * L]],
                    ).partition_broadcast(C),
                )
                sel2s[p] = sel2

            prevs = [None] * NP

            def emit_moment(p, j):
                """One DVE op computing moment j for BOTH taps of pair p,
                then the 16 psum-tile matmuls consuming it."""
                sel2 = sel2s[p]
                if "mult" in ablate:
                    u2w = sel2
                else:
                    u2w = mdpool.tile([C, 2 * L], F16, tag="u")

                    def mul(out, a, b):
                        if "stt" in opts:
                            # InstTensorScalarPtr form of a*b — supports
                            # faster DVE perf modes than InstTensorTensor
                            nc.vector.scalar_tensor_tensor(
                                out=out, in0=a, scalar=1.0, in1=b,
                                op0=OP.mult, op1=OP.mult,
                            )
                        else:
                            nc.vector.tensor_tensor(
                                out=out, in0=a, in1=b, op=OP.mult
                            )

                    if j == 0:
                        for q in range(2):
                            mul(
                                u2w[:, q * L : (q + 1) * L],
                                xvs[2 * p + q],
                                sel2[:, q * L : (q + 1) * L],
                            )
                    else:
                        mul(u2w[:, :], prevs[p][:, :], sel2[:, :])
                    prevs[p] = u2w
                for q in range(2):
                    slot = 2 * p + q
                    emit_matmuls(
                        1 + 3 * slot + j,
                        u2w[:, q * L : (q + 1) * L],
                        None,
                        stop=(slot == NK - 1 and j == 2),
                    )

            # pair-level software pipeline: one independent op between the
            # links of each pair's u1->u2->u3 chain
            emit_bcast(0)
            emit_bcast(1)
            emit_moment(0, 0)
            emit_moment(0, 1)
            for p in range(1, NP):
                if p + 1 < NP:
                    emit_bcast(p + 1)
                emit_moment(p, 0)
                emit_moment(p - 1, 2)
                emit_moment(p, 1)
            emit_moment(NP - 1, 2)

            # ---- evict -------------------------------------------------
            osb = opool.tile([O, L], F16, tag="osb")
            for t in range(nt_eff):
                nc.scalar.activation(
                    out=osb[:, NTW * t : NTW * (t + 1)],
                    in_=psums[t][:, :],
                    func=AF.Copy,
                )
                if nt_eff == NT and t % 4 == 3:
                    nc.sync.dma_start(
                        out=out_d[:, (t - 3) * NTW : (t + 1) * NTW],
                        in_=osb[:, (t - 3) * NTW : (t + 1) * NTW],
                    )
            if nt_eff != NT:
                nc.sync.dma_start(out=out_d[:, :], in_=osb[:, :])

    nc.compile()
    return nc


_NC = None


def _get_program():
    global _NC
    if _NC is None:
        _NC = _build_program()
    return _NC


def _prep_weights(w0, w1, w2):
    # Vandermonde decode for codes (1, -1, 2): V_j = sum_b inv(A)[j,b] W_b
    # with A[a][j] = t_a^(j+1). Slot 0 is the center tap (always branch 1).
    A = np.array([[1, 1, 1], [-1, 1, -1], [2, 4, 8]], np.float64)
    Cf = np.linalg.inv(A)
    ws = (np.asarray(w0, np.float64), np.asarray(w1, np.float64),
          np.asarray(w2, np.float64))
    V = [sum(Cf[j, b] * ws[b] for b in range(3)) for j in range(3)]  # [O,C,3,3]
    wt = np.empty((NMM, C, O), np.float32)
    wt[0] = np.asarray(w1, np.float32)[:, :, 1, 1].T
    for i, k in enumerate(KSO):
        for j in range(3):
            wt[1 + 3 * i + j] = V[j][:, :, k // 3, k % 3].T
    # device layout: [C, NMM*O] so the weight DMA is fully contiguous
    return np.ascontiguousarray(
        wt.transpose(1, 0, 2).reshape(C, NMM * O)
    ).astype(np.float16)


def make_in_maps(inputs):
    x = np.asarray(inputs["x"], np.float32)
    depth = np.asarray(inputs["depth"], np.float32)
    fx = np.asarray(inputs["fx"], np.float32)
    wt = _prep_weights(inputs["w0"], inputs["w1"], inputs["w2"])
    in_maps = []
    for i in range(N):
        dpad = np.zeros((EDGE, EDGE), np.float32)
        dpad[1:65, 1:65] = depth[i, 0]
        # drow[dy][p, :] = dpad[p + dy, :], packed side by side at base 0,
        # with the 0.5/fx column appended
        fxh = np.full((64, 1), np.float32(0.5) / fx[i], np.float32)
        dsb = np.concatenate(
            [dpad[0:64], dpad[1:65], dpad[2:66], fxh], axis=1
        )
        in_maps.append(
            {
                "x_in": np.ascontiguousarray(
                    x[i].reshape(C, L).astype(np.float16)
                ),
                "d_in": np.ascontiguousarray(dsb),
                "w_in": wt,
            }
        )
    return in_maps


def kernel(**inputs):
    nc = _get_program()
    in_maps = make_in_maps(inputs)
    res = run_bass_kernel_spmd(nc, in_maps, core_ids=list(range(N)))
    out = np.stack([res.results[i]["out"] for i in range(N)])
    return out.reshape(N, O, H, W).astype(np.float32)
